# revision 7
# baseline (speedup 1.0000x reference)
"""Trainium2 Bass kernel for nn_EquivariantMultiheadAttention.

Sharding: query-point axis (dim 1) split across 8 cores (16 points each).
Host side repacks inputs into matmul-friendly layouts; device does, per
(b, q, sq) tile of 512 keys:
  - kg-MLP: L1 matmul (K=8) -> SiLU -> block-diag L2 (4x 32x32 tile-packed
    matmuls) -> SiLU -> L3 (zero-padded M=32 matmuls accumulating 16 tiles
    into one dense PSUM bank)
  - ky-MLP: L1 is activation-only (key-term precomputed per batch, query
    term folded into the per-tile SiLU bias), then same L2/L3.
  - logits = silu(o_ky) + silu(o_kg); phase 2 (separate ACT table): exp,
    masked numerator/denominator via tensor_tensor_reduce, normalize,
    residual + query mask.
Final w_out projection happens host-side on the tiny [B,N,S,4] result.
"""
import numpy as np

B, N, S, DG, C, HID, COUT = 2, 128, 4, 8, 4, 32, 8
NCORE = 8
QL = N // NCORE          # 16 query points per core
KEY = N * S              # 512 keys
T = B * QL * S           # 128 tiles per core
GRP = 16                 # tiles per group (packed into one L3 PSUM bank)
NGRP = T // GRP          # 8 groups

_PROG = None             # cached (nc, out_name)


def _pack_globals(inp):
    cf = np.ascontiguousarray(np.asarray(inp["coset_functions"], np.float32))
    mask = np.asarray(inp["mask"]).astype(np.float32)
    kyW1 = np.asarray(inp["ky_W1"], np.float32)
    out = {}
    keyterm = np.zeros((B, 128, KEY), np.float32)
    for c in range(C):
        fk = cf[:, :, :, c].reshape(B, KEY)
        keyterm[:, c * 32:(c + 1) * 32, :] = kyW1[c, :, 0][None, :, None] * fk[:, None, :]
    out["keyterm"] = keyterm
    kgW1 = np.asarray(inp["kg_W1"], np.float32)
    w1g = np.zeros((DG, 128), np.float32)
    for c in range(C):
        w1g[:, c * 32:(c + 1) * 32] = kgW1[c].T
    out["w1g"] = w1g
    for nm, W2 in (("w2y", inp["ky_W2"]), ("w2g", inp["kg_W2"])):
        W2 = np.asarray(W2, np.float32)
        L = np.zeros((128, 32), np.float32)
        for c in range(C):
            L[c * 32:(c + 1) * 32, :] = W2[c].T
        out[nm] = L
    for nm, W3 in (("w3y", inp["ky_W3"]), ("w3g", inp["kg_W3"])):
        W3 = np.asarray(W3, np.float32)
        L = np.zeros((128, 8 * 32), np.float32)
        for s in range(8):
            blk = np.zeros((128, 32), np.float32)
            for c in range(C):
                blk[c * 32:(c + 1) * 32, 4 * s + c] = W3[c, 0, :]
            L[:, 32 * s:32 * s + 32] = blk
        out[nm] = L
    bias128 = np.zeros((128, 4), np.float32)
    bias128[:, 0] = np.asarray(inp["kg_b1"], np.float32).reshape(128)
    bias128[:, 1] = np.asarray(inp["ky_b2"], np.float32).reshape(128)
    bias128[:, 2] = np.asarray(inp["kg_b2"], np.float32).reshape(128)
    out["bias128"] = bias128
    fkeym = np.zeros((B, 64, KEY), np.float32)
    maskf = np.zeros((B, 64, KEY), np.float32)
    mk = mask.reshape(B, KEY)
    for u in range(GRP):
        for c in range(C):
            fkeym[:, 4 * u + c, :] = mk * cf[:, :, :, c].reshape(B, KEY)
            maskf[:, 4 * u + c, :] = mk
    out["fkeym"] = fkeym
    out["maskf"] = maskf
    return out


def _pack_core(core, inp, b3y, b3g):
    g = np.asarray(inp["pairwise_g"], np.float32)
    cf = np.asarray(inp["coset_functions"], np.float32)
    mask = np.asarray(inp["mask"]).astype(np.float32)
    kyW1 = np.asarray(inp["ky_W1"], np.float32)
    kyb1 = np.asarray(inp["ky_b1"], np.float32)
    qs = slice(core * QL, (core + 1) * QL)
    out = {}
    gt = g[:, qs]                                        # [B,QL,N,S,S,DG]
    out["g_t"] = np.ascontiguousarray(gt.transpose(0, 1, 3, 5, 2, 4)).reshape(T, DG, KEY)
    bias = np.zeros((128, T), np.float32)
    cfq = cf[:, qs]                                      # [B,QL,S,C]
    for c in range(C):
        fq = cfq[..., c].reshape(T)
        bias[c * 32:(c + 1) * 32, :] = kyW1[c, :, 1][:, None] * fq[None, :] + kyb1[c][:, None]
    out["bias_ky"] = bias
    small = np.zeros((64, 18), np.float32)
    small[:, 0] = np.tile(b3y, GRP)
    small[:, 1] = np.tile(b3g, GRP)
    for t in range(T):
        b, r = divmod(t, QL * S)
        ql, sq = divmod(r, S)
        gidx, u = divmod(t, GRP)
        for c in range(C):
            small[4 * u + c, 2 + gidx] = cfq[b, ql, sq, c]
            small[4 * u + c, 10 + gidx] = mask[b, core * QL + ql, sq]
    out["small64"] = small
    return out


def _build_program():
    from contextlib import ExitStack
    import concourse.bass as bass
    import concourse.tile as tile
    import concourse.mybir as mybir
    from concourse import bacc
    import bass_rust

    f32 = mybir.dt.float32
    AF = mybir.ActivationFunctionType
    ALU = mybir.AluOpType

    nc = bacc.Bacc("TRN2", target_bir_lowering=False, debug=False,
                   enable_asserts=False, num_devices=NCORE)

    din = {}
    for name, shape in (
        ("g_t", [T, DG, KEY]), ("keyterm", [B, 128, KEY]), ("bias_ky", [128, T]),
        ("w1g", [DG, 128]), ("w2y", [128, 32]), ("w2g", [128, 32]),
        ("w3y", [128, 256]), ("w3g", [128, 256]), ("bias128", [128, 4]),
        ("small64", [64, 18]), ("fkeym", [B, 64, KEY]), ("maskf", [B, 64, KEY]),
    ):
        din[name] = nc.dram_tensor(name, shape, f32, kind="ExternalInput").ap()
    dout = nc.dram_tensor("out64", [64, NGRP], f32, kind="ExternalOutput").ap()

    with tile.TileContext(nc) as tc, ExitStack() as ctx:
        const = ctx.enter_context(tc.tile_pool(name="const", bufs=1))
        work = ctx.enter_context(tc.tile_pool(name="work", bufs=2))
        gp = ctx.enter_context(tc.tile_pool(name="gp", bufs=3))
        ps = ctx.enter_context(tc.tile_pool(name="ps", bufs=1, space="PSUM"))
        ep = ctx.enter_context(tc.tile_pool(name="ep", bufs=2))

        # --- constants to SBUF ---
        keyterm_s = const.tile([128, B * KEY], f32, name="keyterm_s")
        for b in range(B):
            nc.sync.dma_start(keyterm_s[:, b * KEY:(b + 1) * KEY], din["keyterm"][b])
        fkeym_s = const.tile([64, B * KEY], f32, name="fkeym_s")
        maskf_s = const.tile([64, B * KEY], f32, name="maskf_s")
        for b in range(B):
            nc.sync.dma_start(fkeym_s[:, b * KEY:(b + 1) * KEY], din["fkeym"][b])
            nc.sync.dma_start(maskf_s[:, b * KEY:(b + 1) * KEY], din["maskf"][b])
        bias_ky_s = const.tile([128, T], f32, name="bias_ky_s")
        nc.sync.dma_start(bias_ky_s[:], din["bias_ky"][:])
        w1g_s = const.tile([DG, 128], f32, name="w1g_s")
        nc.sync.dma_start(w1g_s[:], din["w1g"][:])
        w2y_s = const.tile([128, 32], f32, name="w2y_s")
        nc.sync.dma_start(w2y_s[:], din["w2y"][:])
        w2g_s = const.tile([128, 32], f32, name="w2g_s")
        nc.sync.dma_start(w2g_s[:], din["w2g"][:])
        w3y_s = const.tile([128, 256], f32, name="w3y_s")
        nc.sync.dma_start(w3y_s[:], din["w3y"][:])
        w3g_s = const.tile([128, 256], f32, name="w3g_s")
        nc.sync.dma_start(w3g_s[:], din["w3g"][:])
        bias128_s = const.tile([128, 4], f32, name="bias128_s")
        nc.sync.dma_start(bias128_s[:], din["bias128"][:])
        small64_s = const.tile([64, 18], f32, name="small64_s")
        nc.sync.dma_start(small64_s[:], din["small64"][:])
        logits_all = const.tile([64, NGRP * KEY], f32, name="logits_all")
        out_s = const.tile([64, NGRP], f32, name="out_s")

        b1kg = bias128_s[:, 0:1]
        b2ky = bias128_s[:, 1:2]
        b2kg = bias128_s[:, 2:3]
        b3ky = small64_s[:, 0:1]
        b3kg = small64_s[:, 1:2]

        last_silu = None
        # ================= phase 1: MLPs -> logits (Silu table) ==========
        for gidx in range(NGRP):
            b = gidx // (NGRP // B)
            psky = ps.tile([64, KEY], f32, tag="ps3ky", name="psky")
            pskg = ps.tile([64, KEY], f32, tag="ps3kg", name="pskg")
            for u in range(GRP):
                t = gidx * GRP + u
                gt = gp.tile([DG, KEY], f32, tag="gt", name="gt")
                nc.sync.dma_start(gt[:], din["g_t"][t])
                ps1 = ps.tile([128, KEY], f32, tag="ps1", bufs=2, name="ps1")
                nc.tensor.matmul(ps1[:], w1g_s[:], gt[:], start=True, stop=True)
                h1kg = work.tile([128, KEY], f32, tag="h1kg", name="h1kg")
                nc.scalar.activation(h1kg[:], ps1[:], AF.Silu, bias=b1kg)
                h1ky = work.tile([128, KEY], f32, tag="h1ky", name="h1ky")
                nc.scalar.activation(h1ky[:], keyterm_s[:, b * KEY:(b + 1) * KEY],
                                     AF.Silu, bias=bias_ky_s[:, t:t + 1])
                ps2y = ps.tile([128, KEY], f32, tag="ps2y", bufs=2, name="ps2y")
                ps2g = ps.tile([128, KEY], f32, tag="ps2g", bufs=2, name="ps2g")
                for c in range(C):
                    sl = slice(32 * c, 32 * c + 32)
                    nc.tensor.matmul(ps2y[sl, :], w2y_s[sl, :], h1ky[sl, :],
                                     start=True, stop=True,
                                     tile_position=(32 * c, 32 * c))
                    nc.tensor.matmul(ps2g[sl, :], w2g_s[sl, :], h1kg[sl, :],
                                     start=True, stop=True,
                                     tile_position=(32 * c, 32 * c))
                h2ky = work.tile([128, KEY], f32, tag="h2ky", name="h2ky")
                nc.scalar.activation(h2ky[:], ps2y[:], AF.Silu, bias=b2ky)
                h2kg = work.tile([128, KEY], f32, tag="h2kg", name="h2kg")
                nc.scalar.activation(h2kg[:], ps2g[:], AF.Silu, bias=b2kg)
                s_, cg = u % 8, u // 8
                nc.tensor.matmul(psky[32 * cg:32 * cg + 32, :],
                                 w3y_s[:, 32 * s_:32 * s_ + 32], h2ky[:],
                                 start=(s_ == 0), stop=(s_ == 7),
                                 tile_position=(0, 32 * cg))
                nc.tensor.matmul(pskg[32 * cg:32 * cg + 32, :],
                                 w3g_s[:, 32 * s_:32 * s_ + 32], h2kg[:],
                                 start=(s_ == 0), stop=(s_ == 7),
                                 tile_position=(0, 32 * cg))
            sky = work.tile([64, KEY], f32, tag="sky", name="sky")
            nc.scalar.activation(sky[:], psky[:], AF.Silu, bias=b3ky)
            skg = work.tile([64, KEY], f32, tag="skg", name="skg")
            h = nc.scalar.activation(skg[:], pskg[:], AF.Silu, bias=b3kg)
            last_silu = h.ins
            nc.vector.tensor_add(logits_all[:, gidx * KEY:(gidx + 1) * KEY],
                                 sky[:], skg[:])

        # ================= phase 2: exp + softmax-aggregate (Exp table) ==
        import os as _os
        use_dep = _os.environ.get("K_NO_DEP", "0") != "1"
        # tensor_tensor_reduce fails at runtime on this PJRT/axon path
        use_ttr = _os.environ.get("K_USE_TTR", "0") == "1"
        for gidx in range(NGRP):
            b = gidx // (NGRP // B)
            e = ep.tile([64, KEY], f32, tag="e", name="e")
            h = nc.scalar.activation(e[:], logits_all[:, gidx * KEY:(gidx + 1) * KEY],
                                     AF.Exp)
            if use_dep:
                bass_rust.add_dep_helper(h.ins, last_silu,
                                         reason="act-table phase barrier")
            scr = ep.tile([64, KEY], f32, tag="scr", name="scr")
            num = ep.tile([64, 1], f32, tag="num", name="num")
            scr2 = ep.tile([64, KEY], f32, tag="scr2", name="scr2")
            den = ep.tile([64, 1], f32, tag="den", name="den")
            if use_ttr:
                nc.vector.tensor_tensor_reduce(
                    out=scr[:], in0=e[:], in1=fkeym_s[:, b * KEY:(b + 1) * KEY],
                    scale=1.0, scalar=0.0, op0=ALU.mult, op1=ALU.add, accum_out=num[:])
                nc.vector.tensor_tensor_reduce(
                    out=scr2[:], in0=e[:], in1=maskf_s[:, b * KEY:(b + 1) * KEY],
                    scale=1.0, scalar=0.0, op0=ALU.mult, op1=ALU.add, accum_out=den[:])
            else:
                nc.vector.tensor_mul(scr[:], e[:], fkeym_s[:, b * KEY:(b + 1) * KEY])
                nc.vector.tensor_reduce(num[:], scr[:], mybir.AxisListType.X, ALU.add)
                nc.vector.tensor_mul(scr2[:], e[:], maskf_s[:, b * KEY:(b + 1) * KEY])
                nc.vector.tensor_reduce(den[:], scr2[:], mybir.AxisListType.X, ALU.add)
            rden = ep.tile([64, 1], f32, tag="rden", name="rden")
            nc.vector.reciprocal(rden[:], den[:])
            agg = ep.tile([64, 1], f32, tag="agg", name="agg")
            nc.vector.tensor_mul(agg[:], num[:], rden[:])
            res = ep.tile([64, 1], f32, tag="res", name="res")
            nc.vector.tensor_add(res[:], agg[:], small64_s[:, 2 + gidx:3 + gidx])
            nc.vector.tensor_mul(out_s[:, gidx:gidx + 1], res[:],
                                 small64_s[:, 10 + gidx:11 + gidx])
        nc.sync.dma_start(dout[:], out_s[:])

    nc.compile()
    return nc


def _get_program():
    global _PROG
    if _PROG is None:
        _PROG = _build_program()
    return _PROG


def kernel(**inputs) -> np.ndarray:
    from concourse.bass_utils import run_bass_kernel_spmd

    inp = {k: np.asarray(v) for k, v in inputs.items()}
    gl = _pack_globals(inp)
    b3y = np.asarray(inp["ky_b3"], np.float32).reshape(C)
    b3g = np.asarray(inp["kg_b3"], np.float32).reshape(C)
    w_out = np.asarray(inp["w_out"], np.float32)

    in_maps = []
    for core in range(NCORE):
        pc = _pack_core(core, inp, b3y, b3g)
        m = dict(gl)
        m.update(pc)
        in_maps.append({k: np.ascontiguousarray(v) for k, v in m.items()})

    nc = _get_program()
    res = run_bass_kernel_spmd(nc, in_maps, core_ids=list(range(NCORE)))

    cf_out = np.zeros((B, N, S, C), np.float32)
    for core in range(NCORE):
        OUT = res.results[core]["out64"]                  # [64, NGRP]
        arr = OUT.reshape(GRP, C, NGRP)                   # [u,c,g]
        arr = arr.transpose(2, 0, 1).reshape(T, C)        # [t, c], t = g*16+u
        arr = arr.reshape(B, QL, S, C)
        cf_out[:, core * QL:(core + 1) * QL] = arr
    return (cf_out @ w_out.T).astype(np.float32)


# revision 14
# speedup vs baseline: 2.4166x; 2.4166x over previous
"""Trainium2 Bass kernel for nn_EquivariantMultiheadAttention.

Sharding: query-point axis (dim 1) split across 8 cores (16 points each).
Host side repacks inputs into matmul-friendly layouts; device does, per
(b, q, sq) tile of 512 keys:
  - kg-MLP: L1 matmul (K=8) -> SiLU -> block-diag L2 (4x 32x32 tile-packed
    matmuls) -> SiLU -> L3 (zero-padded M=32 matmuls accumulating 16 tiles
    into one dense PSUM bank)
  - ky-MLP: L1 is activation-only (key-term precomputed per batch, query
    term folded into the per-tile SiLU bias), then same L2/L3.
  - logits = silu(o_ky) + silu(o_kg); phase 2 (separate ACT table): exp,
    masked numerator/denominator via tensor_tensor_reduce, normalize,
    residual + query mask.
Final w_out projection happens host-side on the tiny [B,N,S,4] result.
"""
import numpy as np
import ml_dtypes

BF16 = ml_dtypes.bfloat16

B, N, S, DG, C, HID, COUT = 2, 128, 4, 8, 4, 32, 8
NCORE = 8
QL = N // NCORE          # 16 query points per core
KEY = N * S              # 512 keys
T = B * QL * S           # 128 tiles per core
GRP = 16                 # tiles per group (packed into one L3 PSUM bank)
NGRP = T // GRP          # 8 groups

_PROG = None             # cached (nc, out_name)


def _pack_globals(inp):
    cf = np.ascontiguousarray(np.asarray(inp["coset_functions"], np.float32))
    mask = np.asarray(inp["mask"]).astype(np.float32)
    kyW1 = np.asarray(inp["ky_W1"], np.float32)
    out = {}
    keyterm = np.zeros((B, 128, KEY), np.float32)
    for c in range(C):
        fk = cf[:, :, :, c].reshape(B, KEY)
        keyterm[:, c * 32:(c + 1) * 32, :] = kyW1[c, :, 0][None, :, None] * fk[:, None, :]
    out["keyterm"] = keyterm
    kgW1 = np.asarray(inp["kg_W1"], np.float32)
    w1g = np.zeros((DG, 128), np.float32)
    for c in range(C):
        w1g[:, c * 32:(c + 1) * 32] = kgW1[c].T
    out["w1g"] = w1g.astype(BF16)
    for nm, W2 in (("w2y", inp["ky_W2"]), ("w2g", inp["kg_W2"])):
        W2 = np.asarray(W2, np.float32)
        L = np.zeros((128, 32), np.float32)
        for c in range(C):
            L[c * 32:(c + 1) * 32, :] = W2[c].T
        out[nm] = L.astype(BF16)
    for nm, W3 in (("w3y", inp["ky_W3"]), ("w3g", inp["kg_W3"])):
        W3 = np.asarray(W3, np.float32)
        L = np.zeros((128, 8 * 32), np.float32)
        for s in range(8):
            blk = np.zeros((128, 32), np.float32)
            for c in range(C):
                blk[c * 32:(c + 1) * 32, 4 * s + c] = W3[c, 0, :]
            L[:, 32 * s:32 * s + 32] = blk
        out[nm] = L.astype(BF16)
    bias128 = np.zeros((128, 4), np.float32)
    bias128[:, 0] = np.asarray(inp["kg_b1"], np.float32).reshape(128)
    bias128[:, 1] = np.asarray(inp["ky_b2"], np.float32).reshape(128)
    bias128[:, 2] = np.asarray(inp["kg_b2"], np.float32).reshape(128)
    out["bias128"] = bias128
    fkeym = np.zeros((B, 64, KEY), np.float32)
    maskf = np.zeros((B, 64, KEY), np.float32)
    mk = mask.reshape(B, KEY)
    for u in range(GRP):
        for c in range(C):
            fkeym[:, 4 * u + c, :] = mk * cf[:, :, :, c].reshape(B, KEY)
            maskf[:, 4 * u + c, :] = mk
    out["fkeym"] = fkeym
    out["maskf"] = maskf
    return out


def _pack_core(core, inp, b3y, b3g):
    g = np.asarray(inp["pairwise_g"], np.float32)
    cf = np.asarray(inp["coset_functions"], np.float32)
    mask = np.asarray(inp["mask"]).astype(np.float32)
    kyW1 = np.asarray(inp["ky_W1"], np.float32)
    kyb1 = np.asarray(inp["ky_b1"], np.float32)
    qs = slice(core * QL, (core + 1) * QL)
    out = {}
    gt = g[:, qs]                                        # [B,QL,N,S,S,DG]
    out["g_t"] = np.ascontiguousarray(
        gt.transpose(0, 1, 3, 5, 2, 4)).reshape(T, DG, KEY).astype(BF16)
    bias = np.zeros((128, T), np.float32)
    cfq = cf[:, qs]                                      # [B,QL,S,C]
    for c in range(C):
        fq = cfq[..., c].reshape(T)
        bias[c * 32:(c + 1) * 32, :] = kyW1[c, :, 1][:, None] * fq[None, :] + kyb1[c][:, None]
    out["bias_ky"] = bias
    small = np.zeros((64, 18), np.float32)
    small[:, 0] = np.tile(b3y, GRP)
    small[:, 1] = np.tile(b3g, GRP)
    for t in range(T):
        b, r = divmod(t, QL * S)
        ql, sq = divmod(r, S)
        gidx, u = divmod(t, GRP)
        for c in range(C):
            small[4 * u + c, 2 + gidx] = cfq[b, ql, sq, c]
            small[4 * u + c, 10 + gidx] = mask[b, core * QL + ql, sq]
    out["small64"] = small
    return out


def _build_program():
    from contextlib import ExitStack
    import concourse.bass as bass
    import concourse.tile as tile
    import concourse.mybir as mybir
    from concourse import bacc
    import bass_rust

    f32 = mybir.dt.float32
    bf16 = mybir.dt.bfloat16
    AF = mybir.ActivationFunctionType
    ALU = mybir.AluOpType

    nc = bacc.Bacc("TRN2", target_bir_lowering=False, debug=False,
                   enable_asserts=False, num_devices=NCORE)

    din = {}
    for name, shape, dt in (
        ("g_t", [T, DG, KEY], bf16), ("keyterm", [B, 128, KEY], f32),
        ("bias_ky", [128, T], f32),
        ("w1g", [DG, 128], bf16), ("w2y", [128, 32], bf16), ("w2g", [128, 32], bf16),
        ("w3y", [128, 256], bf16), ("w3g", [128, 256], bf16),
        ("bias128", [128, 4], f32),
        ("small64", [64, 18], f32), ("fkeym", [B, 64, KEY], f32),
        ("maskf", [B, 64, KEY], f32),
    ):
        din[name] = nc.dram_tensor(name, shape, dt, kind="ExternalInput").ap()
    dout = nc.dram_tensor("out64", [64, NGRP], f32, kind="ExternalOutput").ap()

    with tile.TileContext(nc) as tc, ExitStack() as ctx:
        const = ctx.enter_context(tc.tile_pool(name="const", bufs=1))
        work = ctx.enter_context(tc.tile_pool(name="work", bufs=2))
        gp = ctx.enter_context(tc.tile_pool(name="gp", bufs=3))
        ps = ctx.enter_context(tc.tile_pool(name="ps", bufs=1, space="PSUM"))
        ep = ctx.enter_context(tc.tile_pool(name="ep", bufs=2))

        # --- constants to SBUF ---
        keyterm_s = const.tile([128, B * KEY], f32, name="keyterm_s")
        for b in range(B):
            nc.sync.dma_start(keyterm_s[:, b * KEY:(b + 1) * KEY], din["keyterm"][b])
        fkeym_s = const.tile([64, B * KEY], f32, name="fkeym_s")
        maskf_s = const.tile([64, B * KEY], f32, name="maskf_s")
        for b in range(B):
            nc.sync.dma_start(fkeym_s[:, b * KEY:(b + 1) * KEY], din["fkeym"][b])
            nc.sync.dma_start(maskf_s[:, b * KEY:(b + 1) * KEY], din["maskf"][b])
        bias_ky_s = const.tile([128, T], f32, name="bias_ky_s")
        nc.sync.dma_start(bias_ky_s[:], din["bias_ky"][:])
        w1g_s = const.tile([DG, 128], bf16, name="w1g_s")
        nc.sync.dma_start(w1g_s[:], din["w1g"][:])
        w2y_s = const.tile([128, 32], bf16, name="w2y_s")
        nc.sync.dma_start(w2y_s[:], din["w2y"][:])
        w2g_s = const.tile([128, 32], bf16, name="w2g_s")
        nc.sync.dma_start(w2g_s[:], din["w2g"][:])
        w3y_s = const.tile([128, 256], bf16, name="w3y_s")
        nc.sync.dma_start(w3y_s[:], din["w3y"][:])
        w3g_s = const.tile([128, 256], bf16, name="w3g_s")
        nc.sync.dma_start(w3g_s[:], din["w3g"][:])
        bias128_s = const.tile([128, 4], f32, name="bias128_s")
        nc.sync.dma_start(bias128_s[:], din["bias128"][:])
        small64_s = const.tile([64, 18], f32, name="small64_s")
        nc.sync.dma_start(small64_s[:], din["small64"][:])
        logits_all = const.tile([64, NGRP * KEY], f32, name="logits_all")
        out_s = const.tile([64, NGRP], f32, name="out_s")

        b1kg = bias128_s[:, 0:1]
        b2ky = bias128_s[:, 1:2]
        b2kg = bias128_s[:, 2:3]
        b3ky = small64_s[:, 0:1]
        b3kg = small64_s[:, 1:2]

        last_silu = None
        # ================= phase 1: MLPs -> logits (Silu table) ==========
        # Tiles processed in pairs: psum [128, 2*KEY] spans 2 banks (one per
        # tile), one FD=1024 ACT op covers both.
        for gidx in range(NGRP):
            b = gidx // (NGRP // B)
            psky = ps.tile([64, KEY], f32, tag="ps3ky", name="psky")
            pskg = ps.tile([64, KEY], f32, tag="ps3kg", name="pskg")
            for up in range(GRP // 2):
                t0 = gidx * GRP + 2 * up
                gt = gp.tile([DG, 2 * KEY], bf16, tag="gt", name="gt")
                nc.sync.dma_start(
                    gt[:].rearrange("p (t k) -> p t k", t=2),
                    din["g_t"][t0:t0 + 2].rearrange("t p k -> p t k"))
                ps1 = ps.tile([128, 2 * KEY], f32, tag="ps1", name="ps1")
                for h_ in range(2):
                    nc.tensor.matmul(ps1[:, h_ * KEY:(h_ + 1) * KEY], w1g_s[:],
                                     gt[:, h_ * KEY:(h_ + 1) * KEY],
                                     start=True, stop=True)
                h1kg = work.tile([128, 2 * KEY], bf16, tag="h1kg", name="h1kg")
                nc.scalar.activation(h1kg[:], ps1[:], AF.Silu, bias=b1kg)
                h1ky = work.tile([128, 2 * KEY], bf16, tag="h1ky", name="h1ky")
                for h_ in range(2):
                    nc.scalar.activation(h1ky[:, h_ * KEY:(h_ + 1) * KEY],
                                         keyterm_s[:, b * KEY:(b + 1) * KEY],
                                         AF.Silu, bias=bias_ky_s[:, t0 + h_:t0 + h_ + 1])
                ps2y = ps.tile([128, 2 * KEY], f32, tag="ps2y", name="ps2y")
                ps2g = ps.tile([128, 2 * KEY], f32, tag="ps2g", name="ps2g")
                for h_ in range(2):
                    hs = slice(h_ * KEY, (h_ + 1) * KEY)
                    for c in range(C):
                        sl = slice(32 * c, 32 * c + 32)
                        nc.tensor.matmul(ps2y[sl, hs], w2y_s[sl, :], h1ky[sl, hs],
                                         start=True, stop=True,
                                         tile_position=(32 * c, 32 * c))
                        nc.tensor.matmul(ps2g[sl, hs], w2g_s[sl, :], h1kg[sl, hs],
                                         start=True, stop=True,
                                         tile_position=(32 * c, 32 * c))
                h2ky = work.tile([128, 2 * KEY], bf16, tag="h2ky", name="h2ky")
                nc.scalar.activation(h2ky[:], ps2y[:], AF.Silu, bias=b2ky)
                h2kg = work.tile([128, 2 * KEY], bf16, tag="h2kg", name="h2kg")
                nc.scalar.activation(h2kg[:], ps2g[:], AF.Silu, bias=b2kg)
                for h_ in range(2):
                    u = 2 * up + h_
                    hs = slice(h_ * KEY, (h_ + 1) * KEY)
                    s_, cg = u % 8, u // 8
                    nc.tensor.matmul(psky[32 * cg:32 * cg + 32, :],
                                     w3y_s[:, 32 * s_:32 * s_ + 32], h2ky[:, hs],
                                     start=(s_ == 0), stop=(s_ == 7),
                                     tile_position=(0, 32 * cg))
                    nc.tensor.matmul(pskg[32 * cg:32 * cg + 32, :],
                                     w3g_s[:, 32 * s_:32 * s_ + 32], h2kg[:, hs],
                                     start=(s_ == 0), stop=(s_ == 7),
                                     tile_position=(0, 32 * cg))
            sky = work.tile([64, KEY], f32, tag="sky", name="sky")
            nc.scalar.activation(sky[:], psky[:], AF.Silu, bias=b3ky)
            skg = work.tile([64, KEY], f32, tag="skg", name="skg")
            h = nc.scalar.activation(skg[:], pskg[:], AF.Silu, bias=b3kg)
            last_silu = h.ins
            nc.vector.tensor_add(logits_all[:, gidx * KEY:(gidx + 1) * KEY],
                                 sky[:], skg[:])

        # ================= phase 2: exp + softmax-aggregate (Exp table) ==
        import os as _os
        use_dep = _os.environ.get("K_NO_DEP", "0") != "1"
        # tensor_tensor_reduce fails at runtime on this PJRT/axon path
        use_ttr = _os.environ.get("K_USE_TTR", "0") == "1"
        for gidx in range(NGRP):
            b = gidx // (NGRP // B)
            e = ep.tile([64, KEY], f32, tag="e", name="e")
            h = nc.scalar.activation(e[:], logits_all[:, gidx * KEY:(gidx + 1) * KEY],
                                     AF.Exp)
            if use_dep:
                bass_rust.add_dep_helper(h.ins, last_silu,
                                         reason="act-table phase barrier")
            scr = ep.tile([64, KEY], f32, tag="scr", name="scr")
            num = ep.tile([64, 1], f32, tag="num", name="num")
            scr2 = ep.tile([64, KEY], f32, tag="scr2", name="scr2")
            den = ep.tile([64, 1], f32, tag="den", name="den")
            if use_ttr:
                nc.vector.tensor_tensor_reduce(
                    out=scr[:], in0=e[:], in1=fkeym_s[:, b * KEY:(b + 1) * KEY],
                    scale=1.0, scalar=0.0, op0=ALU.mult, op1=ALU.add, accum_out=num[:])
                nc.vector.tensor_tensor_reduce(
                    out=scr2[:], in0=e[:], in1=maskf_s[:, b * KEY:(b + 1) * KEY],
                    scale=1.0, scalar=0.0, op0=ALU.mult, op1=ALU.add, accum_out=den[:])
            else:
                nc.vector.tensor_mul(scr[:], e[:], fkeym_s[:, b * KEY:(b + 1) * KEY])
                nc.vector.tensor_reduce(num[:], scr[:], mybir.AxisListType.X, ALU.add)
                nc.vector.tensor_mul(scr2[:], e[:], maskf_s[:, b * KEY:(b + 1) * KEY])
                nc.vector.tensor_reduce(den[:], scr2[:], mybir.AxisListType.X, ALU.add)
            rden = ep.tile([64, 1], f32, tag="rden", name="rden")
            nc.vector.reciprocal(rden[:], den[:])
            agg = ep.tile([64, 1], f32, tag="agg", name="agg")
            nc.vector.tensor_mul(agg[:], num[:], rden[:])
            res = ep.tile([64, 1], f32, tag="res", name="res")
            nc.vector.tensor_add(res[:], agg[:], small64_s[:, 2 + gidx:3 + gidx])
            nc.vector.tensor_mul(out_s[:, gidx:gidx + 1], res[:],
                                 small64_s[:, 10 + gidx:11 + gidx])
        nc.sync.dma_start(dout[:], out_s[:])

    nc.compile()
    return nc


def _get_program():
    global _PROG
    if _PROG is None:
        _PROG = _build_program()
    return _PROG


def kernel(**inputs) -> np.ndarray:
    from concourse.bass_utils import run_bass_kernel_spmd

    inp = {k: np.asarray(v) for k, v in inputs.items()}
    gl = _pack_globals(inp)
    b3y = np.asarray(inp["ky_b3"], np.float32).reshape(C)
    b3g = np.asarray(inp["kg_b3"], np.float32).reshape(C)
    w_out = np.asarray(inp["w_out"], np.float32)

    in_maps = []
    for core in range(NCORE):
        pc = _pack_core(core, inp, b3y, b3g)
        m = dict(gl)
        m.update(pc)
        in_maps.append({k: np.ascontiguousarray(v) for k, v in m.items()})

    nc = _get_program()
    res = run_bass_kernel_spmd(nc, in_maps, core_ids=list(range(NCORE)))

    cf_out = np.zeros((B, N, S, C), np.float32)
    for core in range(NCORE):
        OUT = res.results[core]["out64"]                  # [64, NGRP]
        arr = OUT.reshape(GRP, C, NGRP)                   # [u,c,g]
        arr = arr.transpose(2, 0, 1).reshape(T, C)        # [t, c], t = g*16+u
        arr = arr.reshape(B, QL, S, C)
        cf_out[:, core * QL:(core + 1) * QL] = arr
    return (cf_out @ w_out.T).astype(np.float32)
